# revision 1
# baseline (speedup 1.0000x reference)
"""Liquid State Machine kernel for Trainium2, 8 NeuronCores.

Strategy
--------
R=2048 reservoir is split 8 ways (256 neurons per core); batch B=64 and the
time loop stay on every core.  State is kept transposed ([neuron, batch]) so
each core's new spike slice is immediately in the right layout to be the
matmul moving operand of the next step.

Per step t (on each core):
    I_slice = W_rec[:, my 256 cols]^T-contracted with full spikes[t]  (R=2048
              contraction) + W_in[:, my cols] contracted with x_t.
    u = 0.9 u + I ; spikes[t+1] = u >= 1 ; u *= (u < 1) ; acc += spikes

Numerics: weights are split into 3 bf16 terms (hi + mid + lo ~ 24 mantissa
bits, i.e. full fp32).  Spikes/inputs are {0,1} (exact in bf16), products are
exact, PSUM accumulates in fp32.  The reference dynamics were measured to be
bitwise insensitive to f32 summation order (f32 == f64 == chunked-f32), so
this reproduces the reference spike trains.

Cross-core exchange: after each step every core publishes its 256x64 spike
slice (as [128 part x 128 free] bf16) through a per-step ncfw AllGather
(collective_compute) via HBM bounce buffers; the gathered [8*128, 128] result
is DMA'd back into the rank-ordered SBUF slot buffer for the next step's
matmuls.  (A remote_dma SBUF->SBUF broadcast variant was built and validated
in simulation but this toolchain's walrus build cannot codegen the Q7
extended-ISA load_library instruction, so the ncfw path is shipped.)
Spike buffers are double-buffered (even/odd step); all sequencing is done
with monotonic semaphore thresholds per step.
"""

import numpy as np
import ml_dtypes

from contextlib import ExitStack

import concourse.bass as bass
import concourse.mybir as mybir
from concourse import library_config

ALPHA = 0.9
THRESHOLD = 1.0
B, T, D, R = 64, 256, 512, 2048
NCORES = 8
RC = R // NCORES          # 256 neurons per core
NT_REC = 3                # bf16 split terms for W_rec
NT_IN = 3                 # bf16 split terms for W_in
BF = mybir.dt.bfloat16
F32 = mybir.dt.float32
AOT = mybir.AluOpType


def build_lsm_nc(nsteps=T, nt_rec=NT_REC, nt_in=NT_IN):
    nc = bass.Bass(num_devices=NCORES)

    # ---- DRAM I/O (per-core arrays supplied by host) ----
    wr = [nc.dram_tensor(f"wr{i}", [16, 128, RC], BF, kind="ExternalInput")
          for i in range(nt_rec)]
    wi = [nc.dram_tensor(f"wi{i}", [4, 128, RC], BF, kind="ExternalInput")
          for i in range(nt_in)]
    xt = nc.dram_tensor("xt", [nsteps, 128, 256], BF, kind="ExternalInput")
    ident = nc.dram_tensor("ident", [128, 128], BF, kind="ExternalInput")
    out = nc.dram_tensor("out", [128, 128], F32, kind="ExternalOutput")
    bin_ = nc.dram_tensor("bounce_in", [128, 128], BF)
    bout = nc.dram_tensor("bounce_out", [NCORES * 128, 128], BF)

    nc.all_core_barrier()

    with ExitStack() as ctx:
        WR = ctx.enter_context(nc.sbuf_tensor("WR", [128, nt_rec * 16 * RC], BF))
        WI = ctx.enter_context(nc.sbuf_tensor("WI", [128, nt_in * 4 * RC], BF))
        SPK = ctx.enter_context(nc.sbuf_tensor("SPK", [128, 2 * 1024], BF))
        XT = ctx.enter_context(nc.sbuf_tensor("XT", [128, 4 * 256], BF))
        IDENT = ctx.enter_context(nc.sbuf_tensor("IDENT", [128, 128], BF))
        U = ctx.enter_context(nc.sbuf_tensor("U", [128, 128], F32))
        KEEP = ctx.enter_context(nc.sbuf_tensor("KEEP", [128, 128], F32))
        OUTS = ctx.enter_context(nc.sbuf_tensor("OUTS", [128, 128], F32))
        STAGE = ctx.enter_context(nc.sbuf_tensor("STAGE", [128, 128], BF))
        PS0 = ctx.enter_context(nc.psum_tensor("PS0", [128, 512], F32))
        PS1 = ctx.enter_context(nc.psum_tensor("PS1", [128, 512], F32))
        ACCP = ctx.enter_context(nc.psum_tensor("ACCP", [128, 512], F32))
        sems = {}
        for s in ("sem_w sem_wa sem_fin sem_mm sem_u sem_dve sem_init sem_stg sem_spk cc_sem "
                  "sem_x0 sem_x1 sem_x2 sem_x3").split():
            sems[s] = ctx.enter_context(nc.semaphore(s))
        sem_w, sem_wa, sem_fin = sems["sem_w"], sems["sem_wa"], sems["sem_fin"]
        sem_mm, sem_u = sems["sem_mm"], sems["sem_u"]
        sem_xb = [sems[f"sem_x{i}"] for i in range(4)]
        sem_dve, sem_init = sems["sem_dve"], sems["sem_init"]
        sem_stg, sem_spk, cc_sem = sems["sem_stg"], sems["sem_spk"], sems["cc_sem"]
        PS = [PS0, PS1]

        # SBUF layout helpers
        def wr_tile(term, q, mm):          # lhsT [128, 128] for W_rec
            base = (term * 16 + q) * RC + mm * 128
            return WR[:, base:base + 128]

        def wi_tile(term, dd, mm):
            base = (term * 4 + dd) * RC + mm * 128
            return WI[:, base:base + 128]

        def spk_slot(buf, j):              # [128, 128] (two 64-wide kk blocks)
            return SPK[:, buf * 1024 + j * 128: buf * 1024 + j * 128 + 128]

        def spk_rhs(buf, j, kk):           # [128, 64] moving operand
            base = buf * 1024 + j * 128 + kk * 64
            return SPK[:, base:base + 64]

        def xt_rhs(tb, dd):
            return XT[:, tb * 256 + dd * 64: tb * 256 + dd * 64 + 64]

        n_wdma = nt_rec * 16 + nt_in * 4 + 1   # per-tile weight DMAs + ident
        XBUF = 4

        with nc.Block() as block:

            @block.sync
            def _(sync):
                # ident + W_in first (needed at t=0 .. t=1)
                sync.dma_start(IDENT[:, :], ident[:, :]).then_inc(sem_wa, 16)
                for i in range(nt_in):
                    for dd in range(4):
                        sync.dma_start(
                            WI[:, (i * 4 + dd) * RC:(i * 4 + dd + 1) * RC],
                            wi[i][dd, :, :],
                        ).then_inc(sem_wa, 16)
                for i in range(nt_rec):
                    for q in range(16):
                        sync.dma_start(
                            WR[:, (i * 16 + q) * RC:(i * 16 + q + 1) * RC],
                            wr[i][q, :, :],
                        ).then_inc(sem_w, 16)
                # x tile preload + per-step spike bounce / gather DMAs
                for t in range(min(XBUF, nsteps)):
                    sync.dma_start(
                        XT[:, (t % XBUF) * 256:(t % XBUF) * 256 + 256],
                        xt[t, :, :],
                    ).then_inc(sem_xb[t % XBUF], 16)
                for r in range(1, nsteps):
                    sync.wait_ge(sem_u, r)
                    sync.dma_start(bin_[:, :], STAGE[:, :]).then_inc(sem_stg, 16)
                    sync.wait_ge(cc_sem, r)
                    if r >= 2:
                        sync.wait_ge(sem_mm, r - 1)
                    sync.dma_start(
                        SPK[:, (r % 2) * 1024:(r % 2) * 1024 + 1024],
                        bout.ap().rearrange("(j p) n -> p j n", p=128),
                    ).then_inc(sem_spk, 16)
                    t = r + XBUF - 1
                    if t < nsteps:
                        sync.wait_ge(sem_mm, t - XBUF + 1)
                        sync.dma_start(
                            XT[:, (t % XBUF) * 256:(t % XBUF) * 256 + 256],
                            xt[t, :, :],
                        ).then_inc(sem_xb[t % XBUF], 16)
                # final output
                sync.wait_ge(sem_init, 2)
                sync.dma_start(out[:, :], OUTS[:, :]).then_inc(sem_fin, 16)
                sync.wait_ge(sem_fin, 16)

            @block.gpsimd
            def _(gpsimd):
                gpsimd.memset(U[:, :], 0.0)
                gpsimd.memset(OUTS[:, :], 0.0).then_inc(sem_init, 1)
                for r in range(1, nsteps):
                    gpsimd.wait_ge(sem_stg, 16 * r)
                    gpsimd.collective_compute(
                        "AllGather",
                        mybir.AluOpType.bypass,
                        replica_groups=[list(range(NCORES))],
                        ins=[bin_.ap().opt()],
                        outs=[bout.ap().opt()],
                    ).then_inc(cc_sem, 1)

            @block.tensor
            def _(tensor):
                for t in range(nsteps):
                    buf = t % 2
                    ps = PS[buf]
                    if t == 0:
                        tensor.wait_ge(sem_wa, 16 * (nt_in * 4 + 1))
                    if t == 1:
                        tensor.wait_ge(sem_w, 16 * nt_rec * 16)
                    if t >= 2:
                        tensor.wait_ge(sem_dve, t - 1)
                    tensor.wait_ge(sem_xb[t % XBUF], 16 * (t // XBUF + 1))
                    if t >= 1:
                        tensor.wait_ge(sem_u, t)
                        tensor.matmul(
                            ACCP[:, 0:128], IDENT[:, :], STAGE[:, :],
                            start=(t == 1), stop=False, skip_group_check=True,
                        )
                        tensor.wait_ge(sem_spk, 16 * t)
                    for mm in range(2):
                        for dd in range(4):
                            for i in range(nt_in):
                                last_mm = tensor.matmul(
                                    ps[:, mm * 64:mm * 64 + 64],
                                    wi_tile(i, dd, mm),
                                    xt_rhs(t % XBUF, dd),
                                    start=(dd == 0 and i == 0),
                                    stop=(t == 0 and dd == 3 and i == nt_in - 1),
                                )
                        if t >= 1:
                            for j in range(NCORES):
                                for kk in range(2):
                                    q = 2 * j + kk
                                    for i in range(nt_rec):
                                        last_mm = tensor.matmul(
                                            ps[:, mm * 64:mm * 64 + 64],
                                            wr_tile(i, q, mm),
                                            spk_rhs(buf, j, kk),
                                            start=False,
                                            stop=(kk == 1 and j == NCORES - 1
                                                  and i == nt_rec - 1),
                                        )
                    last_mm.then_inc(sem_mm, 1)
                # tail: count spk[nsteps]
                tensor.wait_ge(sem_u, nsteps)
                tensor.matmul(
                    ACCP[:, 0:128], IDENT[:, :], STAGE[:, :],
                    start=False, stop=True, skip_group_check=True,
                ).then_inc(sem_mm, 1)

            @block.vector
            def _(vector):
                vector.wait_ge(sem_init, 1)
                for t in range(nsteps):
                    buf = t % 2
                    nbuf = (t + 1) % 2
                    ps = PS[buf]
                    vector.wait_ge(sem_mm, t + 1)
                    if t >= 2:
                        vector.wait_ge(sem_stg, 16 * (t - 1))
                    vector.tensor_scalar_mul(U[:, :], U[:, :], ALPHA)
                    vector.drain()
                    vector.tensor_add(U[:, :], U[:, :], ps[:, 0:128]).then_inc(
                        sem_dve, 1)
                    vector.drain()
                    vector.tensor_scalar(
                        STAGE[:, :], U[:, :], THRESHOLD, None, AOT.is_ge,
                    ).then_inc(sem_u, 1)
                    vector.tensor_scalar(
                        KEEP[:, :], U[:, :], THRESHOLD, None, AOT.is_lt)
                    vector.drain()
                    vector.tensor_mul(U[:, :], U[:, :], KEEP[:, :])
                    vector.drain()
                # finalize: rates = ACC / T
                vector.wait_ge(sem_mm, nsteps + 1)
                vector.tensor_scalar_mul(
                    OUTS[:, :], ACCP[:, 0:128], 1.0 / nsteps
                ).then_inc(sem_init, 1)

    return nc


# ---------------- host side ----------------

def _split_bf16(w, nterms):
    terms = []
    rem = w.astype(np.float32)
    for _ in range(nterms):
        t = rem.astype(ml_dtypes.bfloat16)
        terms.append(t)
        rem = rem - t.astype(np.float32)
    return terms


def make_in_maps(inputs, W_in, W_rec, nsteps=T, nt_rec=NT_REC, nt_in=NT_IN):
    """Build the per-core input dicts (with XOR-permuted W_rec row blocks)."""
    inputs = np.asarray(inputs, np.float32)
    W_in = np.asarray(W_in, np.float32)
    W_rec = np.asarray(W_rec, np.float32)
    # xt[t, p, dd*64+b] = inputs[b, t, dd*128+p]
    xtr = np.ascontiguousarray(
        inputs.transpose(1, 2, 0)[:nsteps]                  # [T, D, B]
        .reshape(nsteps, 4, 128, B).transpose(0, 2, 1, 3)   # [T, 128, 4, B]
        .reshape(nsteps, 128, 4 * B)
    ).astype(ml_dtypes.bfloat16)
    ident = np.eye(128, dtype=ml_dtypes.bfloat16)
    in_maps = []
    for c in range(NCORES):
        cols = slice(c * RC, (c + 1) * RC)
        wr_terms = _split_bf16(W_rec[:, cols], nt_rec)
        wi_terms = _split_bf16(W_in[:, cols], nt_in)
        m = {"xt": xtr, "ident": ident}
        for i, w in enumerate(wr_terms):
            m[f"wr{i}"] = np.ascontiguousarray(w.reshape(16, 128, RC))
        for i, w in enumerate(wi_terms):
            m[f"wi{i}"] = np.ascontiguousarray(w.reshape(4, 128, RC))
        in_maps.append(m)
    return in_maps


def assemble_output(results, nsteps=T):
    """results: list of per-core {'out': [128,128]} -> firing rate [B, R]."""
    rate = np.zeros((B, R), np.float32)
    for c, res in enumerate(results):
        o = np.asarray(res["out"])          # [p, mm*64+b]
        o = o.reshape(128, 2, B).transpose(1, 0, 2).reshape(RC, B)  # [n, b]
        rate[:, c * RC:(c + 1) * RC] = o.T
    return rate


def kernel(inputs, W_in, W_rec):
    from concourse import bass_utils
    nc = build_lsm_nc()
    in_maps = make_in_maps(inputs, W_in, W_rec)
    res = bass_utils.run_bass_kernel_spmd(nc, in_maps, core_ids=list(range(NCORES)))
    return assemble_output(res.results)



# revision 2
# speedup vs baseline: 1854.9996x; 1854.9996x over previous
"""Liquid State Machine kernel for Trainium2, 8 NeuronCores.

Strategy
--------
R=2048 reservoir split 8 ways (256 neurons per core); batch B=64 and the
time loop stay on every core.  State is kept transposed ([neuron, batch]) so
each core's new spike slice is immediately in the right layout to be the
matmul moving operand of the next step.

Per step t (on each core):
    I_slice = W_in[:, my cols] contracted with x_t   (into PSW bank, computed
              one step AHEAD so it overlaps the spike exchange)
            + W_rec[:, my cols]^T contracted with full spikes[t]  (PSR bank)
    u = alpha*u + I ; spikes[t+1] = u >= 1 ; u *= (u < 1) ; acc += spikes

Numerics: weights split into 2 bf16 terms (~16 mantissa bits).  Spikes and
inputs are {0,1} (exact in bf16), products exact, PSUM accumulates fp32.
Realized rel_err vs the fp32 reference is ~1.37e-2 (deterministic), within
the 2e-2 gate.  (3-term split reproduces fp32 exactly up to an f32 grouping
difference of ~2.5e-3 and runs ~15% slower; 2 terms chosen for speed.)

Cross-core exchange: per-step SBUF->SBUF ncfw AllGather (InstCollectiveCompute
emitted directly with cc_dim="Free"; walrus supports this "SB global CC" even
though the bass helper asserts it off).  Each core's 128x128 bf16 STAGE tile
gathers straight into the double-buffered SPK slot buffer - no HBM bounce,
no gather DMA, no DMA-semaphore hops.  Validated stable and bit-deterministic
across repeated runs.
"""

import numpy as np
import ml_dtypes

from contextlib import ExitStack

import concourse.bass as bass
import concourse.mybir as mybir

ALPHA = 0.9
THRESHOLD = 1.0
B, T, D, R = 64, 256, 512, 2048
NCORES = 8
RC = R // NCORES          # 256 neurons per core
NT_REC = 2                # bf16 split terms for W_rec
NT_IN = 2                 # bf16 split terms for W_in
BF = mybir.dt.bfloat16
F32 = mybir.dt.float32
AOT = mybir.AluOpType


def _cc_sbsb(eng, nc, in_ap, out_ap):
    """SBUF->SBUF AllGather (walrus 'SB global CC', free-dim concat)."""
    return eng.add_instruction(
        mybir.InstCollectiveCompute(
            name=f"I-{nc.next_id()}",
            kind="AllGather",
            op=mybir.AluOpType.bypass,
            replica_groups=[list(range(NCORES))],
            ins=[eng.lower_ap(in_ap)],
            outs=[eng.lower_ap(out_ap)],
            unique_tensors="No",
            cc_dim="Free",
        )
    )


def build_lsm_nc(nsteps=T, xt_steps=T, nt_rec=NT_REC, nt_in=NT_IN,
                 cc_engine="gpsimd"):
    nc = bass.Bass(num_devices=NCORES)
    nc.has_collectives = True

    wr = [nc.dram_tensor(f"wr{i}", [16, 128, RC], BF, kind="ExternalInput")
          for i in range(nt_rec)]
    wi = [nc.dram_tensor(f"wi{i}", [4, 128, RC], BF, kind="ExternalInput")
          for i in range(nt_in)]
    xt = nc.dram_tensor("xt", [xt_steps, 128, 256], BF, kind="ExternalInput")
    ident = nc.dram_tensor("ident", [128, 128], BF, kind="ExternalInput")
    out = nc.dram_tensor("out", [128, 128], F32, kind="ExternalOutput")

    nc.all_core_barrier()

    with ExitStack() as ctx:
        WR = ctx.enter_context(nc.sbuf_tensor("WR", [128, nt_rec * 16 * RC], BF))
        WI = ctx.enter_context(nc.sbuf_tensor("WI", [128, nt_in * 4 * RC], BF))
        SPK = ctx.enter_context(nc.sbuf_tensor("SPK", [128, 2 * 1024], BF))
        XT = ctx.enter_context(nc.sbuf_tensor("XT", [128, 4 * 256], BF))
        IDENT = ctx.enter_context(nc.sbuf_tensor("IDENT", [128, 128], BF))
        U = ctx.enter_context(nc.sbuf_tensor("U", [128, 128], F32))
        OUTS = ctx.enter_context(nc.sbuf_tensor("OUTS", [128, 128], F32))
        STAGE = ctx.enter_context(nc.sbuf_tensor("STAGE", [128, 128], BF))
        PSW0 = ctx.enter_context(nc.psum_tensor("PSW0", [128, 512], F32))
        PSW1 = ctx.enter_context(nc.psum_tensor("PSW1", [128, 512], F32))
        PSR0 = ctx.enter_context(nc.psum_tensor("PSR0", [128, 512], F32))
        PSR1 = ctx.enter_context(nc.psum_tensor("PSR1", [128, 512], F32))
        ACCP = ctx.enter_context(nc.psum_tensor("ACCP", [128, 512], F32))
        sems = {}
        for s in ("sem_w sem_wa sem_fin sem_mm sem_u sem_dve sem_init cc_sem "
                  "sem_x0 sem_x1 sem_x2 sem_x3").split():
            sems[s] = ctx.enter_context(nc.semaphore(s))
        sem_w, sem_wa, sem_fin = sems["sem_w"], sems["sem_wa"], sems["sem_fin"]
        sem_mm, sem_u = sems["sem_mm"], sems["sem_u"]
        sem_xb = [sems[f"sem_x{i}"] for i in range(4)]
        sem_dve, sem_init = sems["sem_dve"], sems["sem_init"]
        cc_sem = sems["cc_sem"]
        PSW = [PSW0, PSW1]
        PSR = [PSR0, PSR1]

        def wr_tile(term, q, mm):          # lhsT [128, 128] for W_rec
            base = (term * 16 + q) * RC + mm * 128
            return WR[:, base:base + 128]

        def wi_tile(term, dd, mm):
            base = (term * 4 + dd) * RC + mm * 128
            return WI[:, base:base + 128]

        def spk_rhs(buf, j, kk):           # [128, 64] moving operand
            base = buf * 1024 + j * 128 + kk * 64
            return SPK[:, base:base + 64]

        def xt_rhs(tb, dd):
            return XT[:, tb * 256 + dd * 64: tb * 256 + dd * 64 + 64]

        XBUF = 4

        with nc.Block() as block:

            @block.sync
            def _(sync):
                sync.dma_start(IDENT[:, :], ident[:, :]).then_inc(sem_wa, 16)
                for i in range(nt_in):
                    for dd in range(4):
                        sync.dma_start(
                            WI[:, (i * 4 + dd) * RC:(i * 4 + dd + 1) * RC],
                            wi[i][dd, :, :],
                        ).then_inc(sem_wa, 16)
                for i in range(nt_rec):
                    for q in range(16):
                        sync.dma_start(
                            WR[:, (i * 16 + q) * RC:(i * 16 + q + 1) * RC],
                            wr[i][q, :, :],
                        ).then_inc(sem_w, 16)
                for t in range(min(XBUF, nsteps)):
                    sync.dma_start(
                        XT[:, (t % XBUF) * 256:(t % XBUF) * 256 + 256],
                        xt[t % xt_steps, :, :],
                    ).then_inc(sem_xb[t % XBUF], 16)
                for r in range(1, nsteps):
                    t = r + XBUF - 1
                    if t < nsteps:
                        sync.wait_ge(sem_mm, max(t - XBUF + 2, 0))
                        sync.dma_start(
                            XT[:, (t % XBUF) * 256:(t % XBUF) * 256 + 256],
                            xt[t % xt_steps, :, :],
                        ).then_inc(sem_xb[t % XBUF], 16)
                sync.wait_ge(sem_init, 2)
                sync.dma_start(out[:, :], OUTS[:, :]).then_inc(sem_fin, 16)
                sync.wait_ge(sem_fin, 16)

            @block.gpsimd
            def _(gpsimd):
                gpsimd.memset(U[:, :], 0.0)
                gpsimd.memset(OUTS[:, :], 0.0).then_inc(sem_init, 1)
                if cc_engine == "gpsimd":
                    for r in range(1, nsteps):
                        gpsimd.wait_ge(sem_u, r)
                        _cc_sbsb(
                            gpsimd, nc, STAGE[:, :],
                            SPK[:, (r % 2) * 1024:(r % 2) * 1024 + 1024],
                        ).then_inc(cc_sem, 1)

            @block.scalar
            def _(scalar):
                if cc_engine == "scalar":
                    for r in range(1, nsteps):
                        scalar.wait_ge(sem_u, r)
                        _cc_sbsb(
                            scalar, nc, STAGE[:, :],
                            SPK[:, (r % 2) * 1024:(r % 2) * 1024 + 1024],
                        ).then_inc(cc_sem, 1)

            @block.tensor
            def _(tensor):
                def do_win(t):
                    last = None
                    for mm in range(2):
                        for dd in range(4):
                            for i in range(nt_in):
                                last = tensor.matmul(
                                    PSW[t % 2][:, mm * 64:mm * 64 + 64],
                                    wi_tile(i, dd, mm),
                                    xt_rhs(t % XBUF, dd),
                                    start=(dd == 0 and i == 0),
                                    stop=(dd == 3 and i == nt_in - 1),
                                )
                    return last

                for t in range(nsteps):
                    buf = t % 2
                    if t == 0:
                        tensor.wait_ge(sem_wa, 16 * (nt_in * 4 + 1))
                        tensor.wait_ge(sem_xb[0], 16)
                        last_mm = do_win(0)
                        if nsteps > 1:
                            tensor.wait_ge(sem_xb[1], 16)
                            last_mm = do_win(1)
                        last_mm.then_inc(sem_mm, 1)
                        continue
                    if t == 1:
                        tensor.wait_ge(sem_w, 16 * nt_rec * 16)
                    # W_in one step ahead: overlaps this step's exchange
                    if t + 1 < nsteps:
                        tensor.wait_ge(sem_dve, t)
                        tensor.wait_ge(sem_xb[(t + 1) % XBUF],
                                       16 * ((t + 1) // XBUF + 1))
                        do_win(t + 1)
                    if t >= 2:
                        tensor.wait_ge(sem_dve, t - 1)
                    tensor.wait_ge(sem_u, t)
                    tensor.matmul(
                        ACCP[:, 0:128], IDENT[:, :], STAGE[:, :],
                        start=(t == 1), stop=False, skip_group_check=True,
                    )
                    tensor.wait_ge(cc_sem, t)
                    for mm in range(2):
                        for j in range(NCORES):
                            for kk in range(2):
                                q = 2 * j + kk
                                for i in range(nt_rec):
                                    last_mm = tensor.matmul(
                                        PSR[t % 2][:, mm * 64:mm * 64 + 64],
                                        wr_tile(i, q, mm),
                                        spk_rhs(buf, j, kk),
                                        start=(j == 0 and kk == 0 and i == 0),
                                        stop=(j == NCORES - 1 and kk == 1
                                              and i == nt_rec - 1),
                                    )
                    last_mm.then_inc(sem_mm, 1)
                tensor.wait_ge(sem_u, nsteps)
                tensor.matmul(
                    ACCP[:, 0:128], IDENT[:, :], STAGE[:, :],
                    start=False, stop=True, skip_group_check=True,
                ).then_inc(sem_mm, 1)

            @block.vector
            def _(vector):
                vector.wait_ge(sem_init, 1)
                for t in range(nsteps):
                    psw = PSW[t % 2]
                    vector.wait_ge(sem_mm, t + 1)
                    if t >= 1:
                        # u = alpha*u + I_rec ; then += I_in
                        vector.scalar_tensor_tensor(
                            U[:, :], U[:, :], ALPHA, PSR[t % 2][:, 0:128],
                            AOT.mult, AOT.add,
                        )
                        vector.drain()
                        vector.tensor_add(U[:, :], U[:, :], psw[:, 0:128]
                                          ).then_inc(sem_dve, 1)
                        vector.drain()
                    else:
                        vector.scalar_tensor_tensor(
                            U[:, :], U[:, :], ALPHA, psw[:, 0:128],
                            AOT.mult, AOT.add,
                        ).then_inc(sem_dve, 1)
                        vector.drain()
                    vector.tensor_scalar(
                        STAGE[:, :], U[:, :], THRESHOLD, None, AOT.is_ge,
                    ).then_inc(sem_u, 1)
                    vector.scalar_tensor_tensor(
                        U[:, :], U[:, :], THRESHOLD, U[:, :],
                        AOT.is_lt, AOT.mult,
                    )
                    vector.drain()
                vector.wait_ge(sem_mm, nsteps + 1)
                vector.tensor_scalar_mul(
                    OUTS[:, :], ACCP[:, 0:128], 1.0 / nsteps
                ).then_inc(sem_init, 1)

    return nc


# ---------------- host side ----------------

def _split_bf16(w, nterms):
    terms = []
    rem = w.astype(np.float32)
    for _ in range(nterms):
        t = rem.astype(ml_dtypes.bfloat16)
        terms.append(t)
        rem = rem - t.astype(np.float32)
    return terms


def make_in_maps(inputs, W_in, W_rec, xt_steps=T, nt_rec=NT_REC, nt_in=NT_IN):
    """Build the per-core input dicts."""
    inputs = np.asarray(inputs, np.float32)
    W_in = np.asarray(W_in, np.float32)
    W_rec = np.asarray(W_rec, np.float32)
    # xt[t, p, dd*64+b] = inputs[b, t, dd*128+p]
    xtr = np.ascontiguousarray(
        inputs.transpose(1, 2, 0)[:xt_steps]                # [T, D, B]
        .reshape(xt_steps, 4, 128, B).transpose(0, 2, 1, 3)  # [T, 128, 4, B]
        .reshape(xt_steps, 128, 4 * B)
    ).astype(ml_dtypes.bfloat16)
    ident = np.eye(128, dtype=ml_dtypes.bfloat16)
    in_maps = []
    for c in range(NCORES):
        cols = slice(c * RC, (c + 1) * RC)
        wr_terms = _split_bf16(W_rec[:, cols], nt_rec)
        wi_terms = _split_bf16(W_in[:, cols], nt_in)
        m = {"xt": xtr, "ident": ident}
        for i, w in enumerate(wr_terms):
            m[f"wr{i}"] = np.ascontiguousarray(w.reshape(16, 128, RC))
        for i, w in enumerate(wi_terms):
            m[f"wi{i}"] = np.ascontiguousarray(w.reshape(4, 128, RC))
        in_maps.append(m)
    return in_maps


def assemble_output(results, nsteps=T):
    """results: list of per-core {'out': [128,128]} -> firing rate [B, R]."""
    rate = np.zeros((B, R), np.float32)
    for c, res in enumerate(results):
        o = np.asarray(res["out"])          # [p, mm*64+b]
        o = o.reshape(128, 2, B).transpose(1, 0, 2).reshape(RC, B)  # [n, b]
        rate[:, c * RC:(c + 1) * RC] = o.T
    return rate


def kernel(inputs, W_in, W_rec):
    from concourse import bass_utils
    nc = build_lsm_nc()
    in_maps = make_in_maps(inputs, W_in, W_rec)
    res = bass_utils.run_bass_kernel_spmd(nc, in_maps, core_ids=list(range(NCORES)))
    return assemble_output(res.results)


# revision 5
# speedup vs baseline: 1908.1318x; 1.0286x over previous
"""Liquid State Machine kernel for Trainium2, 8 NeuronCores.

Strategy
--------
R=2048 reservoir split 8 ways (256 neurons per core); batch B=64 and the
time loop stay on every core.  State is kept transposed ([neuron, batch]) so
each core's new spike slice is immediately in the right layout to be the
matmul moving operand of the next step.

Per step t (on each core):
    I_slice = W_in[:, my cols] contracted with x_t   (into PSW bank, computed
              one step AHEAD so it overlaps the spike exchange)
            + W_rec[:, my cols]^T contracted with full spikes[t]  (PSR bank)
    u = alpha*u + I ; spikes[t+1] = u >= 1 ; u *= (u < 1) ; acc += spikes

Numerics: weights split into 2 bf16 terms (~16 mantissa bits).  Spikes and
inputs are {0,1} (exact in bf16), products exact, PSUM accumulates fp32.
Realized rel_err vs the fp32 reference is ~1.37e-2 (deterministic), within
the 2e-2 gate.  (3-term split reproduces fp32 exactly up to an f32 grouping
difference of ~2.5e-3 and runs ~15% slower; 2 terms chosen for speed.)

Cross-core exchange: per-step SBUF->SBUF ncfw AllGather (InstCollectiveCompute
emitted directly with cc_dim="Free"; walrus supports this "SB global CC" even
though the bass helper asserts it off).  Each core's 128x128 bf16 STAGE tile
gathers straight into the double-buffered SPK slot buffer - no HBM bounce,
no gather DMA, no DMA-semaphore hops.  Spikes travel as fp8e4m3 ({0,1} exact;
the PE accepts bf16-weight x fp8-spike matmuls), halving the collective
payload to 16KB.  Validated stable and bit-deterministic across repeated
runs.
"""

import numpy as np
import ml_dtypes

from contextlib import ExitStack

import concourse.bass as bass
import concourse.mybir as mybir

ALPHA = 0.9
THRESHOLD = 1.0
B, T, D, R = 64, 256, 512, 2048
NCORES = 8
RC = R // NCORES          # 256 neurons per core
NT_REC = 2                # bf16 split terms for W_rec
NT_IN = 2                 # bf16 split terms for W_in
BF = mybir.dt.bfloat16
F8 = mybir.dt.float8e4
F32 = mybir.dt.float32
AOT = mybir.AluOpType


def _cc_sbsb(eng, nc, in_ap, out_ap):
    """SBUF->SBUF AllGather (walrus 'SB global CC', free-dim concat)."""
    return eng.add_instruction(
        mybir.InstCollectiveCompute(
            name=f"I-{nc.next_id()}",
            kind="AllGather",
            op=mybir.AluOpType.bypass,
            replica_groups=[list(range(NCORES))],
            ins=[eng.lower_ap(in_ap)],
            outs=[eng.lower_ap(out_ap)],
            unique_tensors="No",
            cc_dim="Free",
        )
    )


def build_lsm_nc(nsteps=T, xt_steps=T, nt_rec=NT_REC, nt_in=NT_IN,
                 cc_engine="gpsimd", spk_dtype=F8):
    nc = bass.Bass(num_devices=NCORES)
    nc.has_collectives = True

    wr = [nc.dram_tensor(f"wr{i}", [16, 128, RC], BF, kind="ExternalInput")
          for i in range(nt_rec)]
    wi = [nc.dram_tensor(f"wi{i}", [4, 128, RC], BF, kind="ExternalInput")
          for i in range(nt_in)]
    xt = nc.dram_tensor("xt", [xt_steps, 128, 256], BF, kind="ExternalInput")
    ident = nc.dram_tensor("ident", [128, 128], BF, kind="ExternalInput")
    out = nc.dram_tensor("out", [128, 128], F32, kind="ExternalOutput")

    nc.all_core_barrier()

    with ExitStack() as ctx:
        WR = ctx.enter_context(nc.sbuf_tensor("WR", [128, nt_rec * 16 * RC], BF))
        WI = ctx.enter_context(nc.sbuf_tensor("WI", [128, nt_in * 4 * RC], BF))
        SPK = ctx.enter_context(nc.sbuf_tensor("SPK", [128, 2 * 1024], spk_dtype))
        XT = ctx.enter_context(nc.sbuf_tensor("XT", [128, 4 * 256], BF))
        IDENT = ctx.enter_context(nc.sbuf_tensor("IDENT", [128, 128], BF))
        U = ctx.enter_context(nc.sbuf_tensor("U", [128, 128], F32))
        OUTS = ctx.enter_context(nc.sbuf_tensor("OUTS", [128, 128], F32))
        STAGE = ctx.enter_context(nc.sbuf_tensor("STAGE", [128, 128], spk_dtype))
        PSW0 = ctx.enter_context(nc.psum_tensor("PSW0", [128, 512], F32))
        PSW1 = ctx.enter_context(nc.psum_tensor("PSW1", [128, 512], F32))
        PSR0 = ctx.enter_context(nc.psum_tensor("PSR0", [128, 512], F32))
        PSR1 = ctx.enter_context(nc.psum_tensor("PSR1", [128, 512], F32))
        ACCP = ctx.enter_context(nc.psum_tensor("ACCP", [128, 512], F32))
        sems = {}
        for s in ("sem_w sem_wa sem_fin sem_mm sem_u sem_dve sem_init cc_sem "
                  "sem_x0 sem_x1 sem_x2 sem_x3").split():
            sems[s] = ctx.enter_context(nc.semaphore(s))
        sem_w, sem_wa, sem_fin = sems["sem_w"], sems["sem_wa"], sems["sem_fin"]
        sem_mm, sem_u = sems["sem_mm"], sems["sem_u"]
        sem_xb = [sems[f"sem_x{i}"] for i in range(4)]
        sem_dve, sem_init = sems["sem_dve"], sems["sem_init"]
        cc_sem = sems["cc_sem"]
        PSW = [PSW0, PSW1]
        PSR = [PSR0, PSR1]

        def wr_tile(term, q, mm):          # lhsT [128, 128] for W_rec
            base = (term * 16 + q) * RC + mm * 128
            return WR[:, base:base + 128]

        def wi_tile(term, dd, mm):
            base = (term * 4 + dd) * RC + mm * 128
            return WI[:, base:base + 128]

        def spk_rhs(buf, j, kk):           # [128, 64] moving operand
            base = buf * 1024 + j * 128 + kk * 64
            return SPK[:, base:base + 64]

        def xt_rhs(tb, dd):
            return XT[:, tb * 256 + dd * 64: tb * 256 + dd * 64 + 64]

        XBUF = 4

        with nc.Block() as block:

            @block.sync
            def _(sync):
                sync.dma_start(IDENT[:, :], ident[:, :]).then_inc(sem_wa, 16)
                for i in range(nt_in):
                    for dd in range(4):
                        sync.dma_start(
                            WI[:, (i * 4 + dd) * RC:(i * 4 + dd + 1) * RC],
                            wi[i][dd, :, :],
                        ).then_inc(sem_wa, 16)
                for i in range(nt_rec):
                    for q in range(16):
                        sync.dma_start(
                            WR[:, (i * 16 + q) * RC:(i * 16 + q + 1) * RC],
                            wr[i][q, :, :],
                        ).then_inc(sem_w, 16)
                for t in range(min(XBUF, nsteps)):
                    sync.dma_start(
                        XT[:, (t % XBUF) * 256:(t % XBUF) * 256 + 256],
                        xt[t % xt_steps, :, :],
                    ).then_inc(sem_xb[t % XBUF], 16)
                for r in range(1, nsteps):
                    if cc_engine == "sync":
                        sync.wait_ge(sem_u, r)
                        _cc_sbsb(
                            sync, nc, STAGE[:, :],
                            SPK[:, (r % 2) * 1024:(r % 2) * 1024 + 1024],
                        ).then_inc(cc_sem, 1)
                    t = r + XBUF - 1
                    if t < nsteps:
                        sync.wait_ge(sem_mm, max(t - XBUF + 2, 0))
                        sync.dma_start(
                            XT[:, (t % XBUF) * 256:(t % XBUF) * 256 + 256],
                            xt[t % xt_steps, :, :],
                        ).then_inc(sem_xb[t % XBUF], 16)
                sync.wait_ge(sem_init, 2)
                sync.dma_start(out[:, :], OUTS[:, :]).then_inc(sem_fin, 16)
                sync.wait_ge(sem_fin, 16)

            @block.gpsimd
            def _(gpsimd):
                gpsimd.memset(U[:, :], 0.0)
                gpsimd.memset(OUTS[:, :], 0.0).then_inc(sem_init, 1)
                if cc_engine == "gpsimd":
                    for r in range(1, nsteps):
                        gpsimd.wait_ge(sem_u, r)
                        _cc_sbsb(
                            gpsimd, nc, STAGE[:, :],
                            SPK[:, (r % 2) * 1024:(r % 2) * 1024 + 1024],
                        ).then_inc(cc_sem, 1)

            @block.scalar
            def _(scalar):
                if cc_engine == "scalar":
                    for r in range(1, nsteps):
                        scalar.wait_ge(sem_u, r)
                        _cc_sbsb(
                            scalar, nc, STAGE[:, :],
                            SPK[:, (r % 2) * 1024:(r % 2) * 1024 + 1024],
                        ).then_inc(cc_sem, 1)

            @block.tensor
            def _(tensor):
                def do_win(t):
                    last = None
                    for mm in range(2):
                        for dd in range(4):
                            for i in range(nt_in):
                                last = tensor.matmul(
                                    PSW[t % 2][:, mm * 64:mm * 64 + 64],
                                    wi_tile(i, dd, mm),
                                    xt_rhs(t % XBUF, dd),
                                    start=(dd == 0 and i == 0),
                                    stop=(dd == 3 and i == nt_in - 1),
                                )
                    return last

                for t in range(nsteps):
                    buf = t % 2
                    if t == 0:
                        tensor.wait_ge(sem_wa, 16 * (nt_in * 4 + 1))
                        tensor.wait_ge(sem_xb[0], 16)
                        last_mm = do_win(0)
                        if nsteps > 1:
                            tensor.wait_ge(sem_xb[1], 16)
                            last_mm = do_win(1)
                        last_mm.then_inc(sem_mm, 1)
                        continue
                    if t == 1:
                        tensor.wait_ge(sem_w, 16 * nt_rec * 16)
                    # W_in one step ahead: overlaps this step's exchange
                    if t + 1 < nsteps:
                        tensor.wait_ge(sem_dve, t)
                        tensor.wait_ge(sem_xb[(t + 1) % XBUF],
                                       16 * ((t + 1) // XBUF + 1))
                        do_win(t + 1)
                    if t >= 2:
                        tensor.wait_ge(sem_dve, t - 1)
                    tensor.wait_ge(sem_u, t)
                    tensor.matmul(
                        ACCP[:, 0:128], IDENT[:, :], STAGE[:, :],
                        start=(t == 1), stop=False, skip_group_check=True,
                    )
                    tensor.wait_ge(cc_sem, t)
                    for mm in range(2):
                        for j in range(NCORES):
                            for kk in range(2):
                                q = 2 * j + kk
                                for i in range(nt_rec):
                                    last_mm = tensor.matmul(
                                        PSR[t % 2][:, mm * 64:mm * 64 + 64],
                                        wr_tile(i, q, mm),
                                        spk_rhs(buf, j, kk),
                                        start=(j == 0 and kk == 0 and i == 0),
                                        stop=(j == NCORES - 1 and kk == 1
                                              and i == nt_rec - 1),
                                    )
                    last_mm.then_inc(sem_mm, 1)
                tensor.wait_ge(sem_u, nsteps)
                tensor.matmul(
                    ACCP[:, 0:128], IDENT[:, :], STAGE[:, :],
                    start=False, stop=True, skip_group_check=True,
                ).then_inc(sem_mm, 1)

            @block.vector
            def _(vector):
                vector.wait_ge(sem_init, 1)
                for t in range(nsteps):
                    psw = PSW[t % 2]
                    vector.wait_ge(sem_mm, t + 1)
                    if t >= 1:
                        # u = alpha*u + I_rec ; then += I_in
                        vector.scalar_tensor_tensor(
                            U[:, :], U[:, :], ALPHA, PSR[t % 2][:, 0:128],
                            AOT.mult, AOT.add,
                        )
                        vector.drain()
                        vector.tensor_add(U[:, :], U[:, :], psw[:, 0:128]
                                          ).then_inc(sem_dve, 1)
                        vector.drain()
                    else:
                        vector.scalar_tensor_tensor(
                            U[:, :], U[:, :], ALPHA, psw[:, 0:128],
                            AOT.mult, AOT.add,
                        ).then_inc(sem_dve, 1)
                        vector.drain()
                    vector.tensor_scalar(
                        STAGE[:, :], U[:, :], THRESHOLD, None, AOT.is_ge,
                    ).then_inc(sem_u, 1)
                    vector.scalar_tensor_tensor(
                        U[:, :], U[:, :], THRESHOLD, U[:, :],
                        AOT.is_lt, AOT.mult,
                    )
                    vector.drain()
                vector.wait_ge(sem_mm, nsteps + 1)
                vector.tensor_scalar_mul(
                    OUTS[:, :], ACCP[:, 0:128], 1.0 / nsteps
                ).then_inc(sem_init, 1)

    return nc


# ---------------- host side ----------------

def _split_bf16(w, nterms):
    terms = []
    rem = w.astype(np.float32)
    for _ in range(nterms):
        t = rem.astype(ml_dtypes.bfloat16)
        terms.append(t)
        rem = rem - t.astype(np.float32)
    return terms


def make_in_maps(inputs, W_in, W_rec, xt_steps=T, nt_rec=NT_REC, nt_in=NT_IN):
    """Build the per-core input dicts."""
    inputs = np.asarray(inputs, np.float32)
    W_in = np.asarray(W_in, np.float32)
    W_rec = np.asarray(W_rec, np.float32)
    # xt[t, p, dd*64+b] = inputs[b, t, dd*128+p]
    xtr = np.ascontiguousarray(
        inputs.transpose(1, 2, 0)[:xt_steps]                # [T, D, B]
        .reshape(xt_steps, 4, 128, B).transpose(0, 2, 1, 3)  # [T, 128, 4, B]
        .reshape(xt_steps, 128, 4 * B)
    ).astype(ml_dtypes.bfloat16)
    ident = np.eye(128, dtype=ml_dtypes.bfloat16)
    in_maps = []
    for c in range(NCORES):
        cols = slice(c * RC, (c + 1) * RC)
        wr_terms = _split_bf16(W_rec[:, cols], nt_rec)
        wi_terms = _split_bf16(W_in[:, cols], nt_in)
        m = {"xt": xtr, "ident": ident}
        for i, w in enumerate(wr_terms):
            m[f"wr{i}"] = np.ascontiguousarray(w.reshape(16, 128, RC))
        for i, w in enumerate(wi_terms):
            m[f"wi{i}"] = np.ascontiguousarray(w.reshape(4, 128, RC))
        in_maps.append(m)
    return in_maps


def assemble_output(results, nsteps=T):
    """results: list of per-core {'out': [128,128]} -> firing rate [B, R]."""
    rate = np.zeros((B, R), np.float32)
    for c, res in enumerate(results):
        o = np.asarray(res["out"])          # [p, mm*64+b]
        o = o.reshape(128, 2, B).transpose(1, 0, 2).reshape(RC, B)  # [n, b]
        rate[:, c * RC:(c + 1) * RC] = o.T
    return rate


def kernel(inputs, W_in, W_rec):
    from concourse import bass_utils
    nc = build_lsm_nc()
    in_maps = make_in_maps(inputs, W_in, W_rec)
    res = bass_utils.run_bass_kernel_spmd(nc, in_maps, core_ids=list(range(NCORES)))
    return assemble_output(res.results)
